# revision 5
# baseline (speedup 1.0000x reference)
"""AFNO2D Trainium kernel (v3 — contiguous layouts, batched DMA).

Same stage chain as v2 (ds-A, ds-B, ws-MLP1, ds-MLP2, ws-invH, XBAR-T4,
ws-invW) but every PSUM->SBUF evacuation is a contiguous copy (no
rearranged/transposed access patterns), DMAs are batched (1 load per
batch, 1 store per (batch, j)), and the softshrink is split
ACT-copy + DVE-clamp/sub to balance engines.

Layouts (per core, c=96 channels, b=4 batches):
  x1 [90w, 2j, 90h, 96c]                  input, resident for residual
  xB [90h, 96c, 92k2]                     after W-rfft (stage A)
  zM [96c, 2ri, 46k, 90u]                 after H-fft  (stage B)
  h1 [97c', 2ri, 46k, 90u]                after MLP1 (row 96 = ones)
  q  [90u, 92k2, 96c]                     after MLP2+softshrink
  r4 [90h', 96c, 128k2pad]                after invH
  xt4 [128k2, 8640(h'c)]                  after XBAR transpose
  yt [b, 2j, 90w, 8640(hc)]               output
"""

import sys

sys.path.insert(0, "/opt/trn_rl_repo")

import numpy as np
import ml_dtypes

import concourse.bass as bass
import concourse.mybir as mybir
import concourse.tile as tile
from concourse import bacc
from concourse.bass_utils import run_bass_kernel_spmd

BF16 = mybir.dt.bfloat16
F32 = mybir.dt.float32
FP8 = mybir.dt.float8e4
AF = mybir.ActivationFunctionType
ALU = mybir.AluOpType

# problem constants
B, H, W, C = 4, 90, 180, 768
NB, BS = 8, 96  # blocks, block size (= per-core channels)
KEEP = 46       # kept W-modes
K2 = 92         # 2*KEEP (re,im stacked)
LAM = 0.01      # softshrink lambda
UK = H * KEEP   # 4140 flat (k,u) free size
NM = 460        # MLP1 chunk
HC = H * BS     # 8640

NCORES = 8

# stage-isolation mask for benchmarking (set via kernel2.STAGE_MASK before
# first _get_nc(); None = all stages)
STAGE_MASK = None


def _dft_mats():
    """Host-side DFT matrices (float32, cast to bf16 at upload)."""
    w = np.arange(W)[:, None]
    k = np.arange(KEEP)[None, :]
    ang = 2.0 * np.pi * w * k / W
    fw = np.concatenate([np.cos(ang), -np.sin(ang)], axis=1) / np.sqrt(W)  # [180, 92]
    fw = fw.reshape(2, 90, K2).transpose(1, 0, 2)  # [90w, 2j, 92]

    h = np.arange(H)[:, None]
    u = np.arange(H)[None, :]
    angh = 2.0 * np.pi * h * u / H
    fhc = np.cos(angh) / np.sqrt(H)
    fhs = np.sin(angh) / np.sqrt(H)

    kk = np.arange(KEEP)[:, None]
    ww = np.arange(W)[None, :]
    angw = 2.0 * np.pi * kk * ww / W
    m = np.full((KEEP, 1), 2.0)
    m[0, 0] = 1.0
    gw = np.concatenate(
        [m * np.cos(angw), -m * np.sin(angw)], axis=0
    ) / np.sqrt(W)  # [92, 180]

    bf = ml_dtypes.bfloat16
    return (fw.astype(bf), fhc.astype(np.float32), fhs.astype(np.float32),
            gw.astype(bf))


def _build():
    nc = bacc.Bacc("TRN2", target_bir_lowering=False, debug=False,
                   num_devices=NCORES)

    # DRAM I/O (per core)
    xt = nc.dram_tensor("xt", [B, 90, 2, 90, BS], BF16,
                        kind="ExternalInput").ap()      # [b, w, j, h, c]
    fw_d = nc.dram_tensor("fw", [90, 2, K2], BF16, kind="ExternalInput").ap()
    fhb_d = nc.dram_tensor("fhb", [4, H, H], FP8, kind="ExternalInput").ap()
    fhg_d = nc.dram_tensor("fhg", [3, H, H], BF16, kind="ExternalInput").ap()
    gw_d = nc.dram_tensor("gw", [K2, W], BF16, kind="ExternalInput").ap()
    mw1_d = nc.dram_tensor("mw1", [BS, 3, BS], BF16, kind="ExternalInput").ap()
    w2a_d = nc.dram_tensor("w2a", [BS + 1, 4, BS], FP8,
                           kind="ExternalInput").ap()
    bias1_d = nc.dram_tensor("bias1", [BS, 2], F32, kind="ExternalInput").ap()
    yt = nc.dram_tensor("yt", [B, 2, 90, HC], BF16,
                        kind="ExternalOutput").ap()     # [b, j, w, (h c)]

    with tile.TileContext(nc) as tc:
        wpool = tc.alloc_tile_pool(name="w", bufs=1)
        sb = tc.alloc_tile_pool(name="sb", bufs=1)
        ps = tc.alloc_tile_pool(name="ps", bufs=8, space="PSUM")
        dr = tc.alloc_tile_pool(name="dr", bufs=2, space="DRAM")

        # ---- weights to SBUF (once) ----
        fw_t = wpool.tile([90, 2, K2], BF16, tag="fw")
        nc.gpsimd.dma_start(out=fw_t, in_=fw_d)
        fhb_t = wpool.tile([H, 4, H], FP8, tag="fhb")
        nc.gpsimd.dma_start(out=fhb_t, in_=fhb_d.rearrange("j p m -> p j m"))
        fhg_t = wpool.tile([H, 3, H], BF16, tag="fhg")
        nc.gpsimd.dma_start(out=fhg_t, in_=fhg_d.rearrange("j p m -> p j m"))
        gw_t = wpool.tile([K2, W], BF16, tag="gw")
        nc.gpsimd.dma_start(out=gw_t, in_=gw_d)
        mw1_t = wpool.tile([BS, 3, BS], BF16, tag="mw1")
        nc.gpsimd.dma_start(out=mw1_t, in_=mw1_d)
        w2a_t = wpool.tile([BS + 1, 4, BS], FP8, tag="w2a")
        nc.gpsimd.dma_start(out=w2a_t, in_=w2a_d)
        bias1_t = wpool.tile([BS, 2], F32, tag="bias1")
        nc.gpsimd.dma_start(out=bias1_t, in_=bias1_d)

        ghc, ghs, ghsn = fhg_t[:, 0], fhg_t[:, 1], fhg_t[:, 2]
        w1r, w1i, w1in = mw1_t[:, 0], mw1_t[:, 1], mw1_t[:, 2]

        # ---- persistent activation tiles ----
        h1 = wpool.tile([BS + 1, 2, KEEP, 128], FP8, tag="h1", name="h1")
        nc.gpsimd.memset(h1[:, :, :, H:128], 0.0)
        nc.gpsimd.memset(h1[BS:BS + 1, :, :, 0:H], 1.0)
        xBp = [wpool.tile([H, K2, 128], FP8, tag=f"xB{i}", name=f"xB{i}")
               for i in range(2)]
        for i in range(2):
            nc.gpsimd.memset(xBp[i][:, :, BS:128], 0.0)
        # r4: invH output, padded to 128 k2-cols for the XBAR transpose
        r4 = wpool.tile([H, BS, 128], BF16, tag="r4", name="r4")
        nc.gpsimd.memset(r4[:, :, K2:128], 0.0)
        xt4 = wpool.tile([128, HC], BF16, tag="xt4", name="xt4")

        def make_stages(b):
            st = {}

            def s0():  # load x (one DMA; resident for residual)
                x1 = st["x1"] = sb.tile([90, 2, 90, BS], BF16, tag="x1",
                                        name="x1", bufs=2)
                nc.sync.dma_start(out=x1, in_=xt[b])

            def s1():  # stage A: W-rfft (data-stationary) -> xB [h, k2, c]
                x1 = st["x1"]
                xB = st["xB"] = xBp[b % 2]
                for g in range(20):  # 20 groups of <=5 c
                    c0 = g * 5
                    n = min(5, BS - c0)
                    psA = ps.tile([H, 5, K2], F32, tag="ps", name="psA")
                    for i in range(n):
                        nc.tensor.matmul(psA[:, i, :],
                                         lhsT=x1[:, 0, :, c0 + i],
                                         rhs=fw_t[:, 0, :],
                                         start=True, stop=False)
                        nc.tensor.matmul(psA[:, i, :],
                                         lhsT=x1[:, 1, :, c0 + i],
                                         rhs=fw_t[:, 1, :],
                                         start=False, stop=True)
                    eng = nc.scalar if g % 2 == 0 else nc.vector
                    (eng.copy if g % 2 == 0 else eng.tensor_copy)(
                        out=xB[:, :, c0:c0 + n],
                        in_=psA[:, 0:n, :].rearrange("p c k -> p k c"))

            def s2():  # stage B: H-fft (data-stationary) -> zM [c, ri, k, u]
                xB = st["xB"]
                zM = st["zM"] = sb.tile([BS, KEEP, 2, H], BF16, tag="zM",
                                        name="zM")
                for g in range(KEEP // 2):  # 23 groups of 2 k
                    k0 = g * 2
                    psR = ps.tile([128, 2, 2 * H], F32, tag="ps",
                                  name="psR")
                    for i in range(2):
                        k = k0 + i
                        nc.tensor.matmul(psR[:, i, :], lhsT=xB[:, k, :],
                                         rhs=fhb_t[:, 0:2, :],
                                         start=True, stop=False)
                        nc.tensor.matmul(psR[:, i, :],
                                         lhsT=xB[:, KEEP + k, :],
                                         rhs=fhb_t[:, 2:4, :],
                                         start=False, stop=True)
                    if g % 2 == 0:
                        nc.scalar.copy(out=zM[:, k0:k0 + 2, :, :],
                                       in_=psR[0:BS, :, :])
                    else:
                        nc.vector.tensor_copy(out=zM[:, k0:k0 + 2, :, :],
                                              in_=psR[0:BS, :, :])

            def s3():  # MLP layer 1 (weight-stationary) -> h1 [97, ri, k, u]
                zM = st["zM"]
                for t in range(10):  # k-chunks of <=5 (46 = 9*5 + 1)
                    k0 = t * 5
                    nk = min(5, KEEP - k0)
                    zr = zM[:, k0:k0 + nk, 0, :]
                    zi = zM[:, k0:k0 + nk, 1, :]
                    p1r = ps.tile([BS, 5, H], F32, tag="ps", name="p1r")
                    p1i = ps.tile([BS, 5, H], F32, tag="ps", name="p1i")
                    nc.tensor.matmul(p1r[:, 0:nk, :], lhsT=w1r, rhs=zr,
                                     start=True, stop=False)
                    nc.tensor.matmul(p1i[:, 0:nk, :], lhsT=w1r, rhs=zi,
                                     start=True, stop=False)
                    nc.tensor.matmul(p1r[:, 0:nk, :], lhsT=w1in, rhs=zi,
                                     start=False, stop=True)
                    nc.tensor.matmul(p1i[:, 0:nk, :], lhsT=w1i, rhs=zr,
                                     start=False, stop=True)
                    nc.scalar.activation(h1[0:BS, 0, k0:k0 + nk, 0:H],
                                         p1r[:, 0:nk, :],
                                         AF.Relu, bias=bias1_t[:, 0:1])
                    nc.scalar.activation(h1[0:BS, 1, k0:k0 + nk, 0:H],
                                         p1i[:, 0:nk, :],
                                         AF.Relu, bias=bias1_t[:, 1:2])

            def s4():  # MLP layer 2 (data-stationary) + softshrink -> q
                q = st["q"] = sb.tile([H, KEEP, 2, BS], BF16, tag="q",
                                      name="q")
                for g in range(KEEP // 2):  # 23 groups of 2 k
                    k0 = g * 2
                    qp = ps.tile([128, 2, 2 * BS], F32, tag="ps",
                                 name="qp")
                    for i in range(2):
                        k = k0 + i
                        nc.tensor.matmul(qp[:, i, :], lhsT=h1[:, 0, k, :],
                                         rhs=w2a_t[:, 0:2, :],
                                         start=True, stop=False)
                        nc.tensor.matmul(qp[:, i, :], lhsT=h1[:, 1, k, :],
                                         rhs=w2a_t[:, 2:4, :],
                                         start=False, stop=True)
                    sq = sb.tile([H, 2, 2 * BS], BF16, tag="sq",
                                 name="sq", bufs=2)
                    nc.scalar.copy(out=sq, in_=qp[0:H, :, :])
                    tcl = sb.tile([H, 2, 2 * BS], BF16, tag="tcl",
                                  name="tcl", bufs=2)
                    nc.vector.tensor_scalar(
                        out=tcl, in0=sq,
                        scalar1=-LAM, scalar2=LAM,
                        op0=ALU.max, op1=ALU.min)
                    nc.vector.tensor_tensor(
                        out=q[:, k0:k0 + 2, :, :],
                        in0=sq, in1=tcl,
                        op=ALU.subtract)

            def s5():  # invH (weight-stationary) -> r4 [h', c, k2pad]
                q = st["q"]
                for t in range(10):  # c-groups of <=10
                    c0 = t * 10
                    n = min(10, BS - c0)
                    nf = n * KEEP
                    qr = q[:, :, 0, c0:c0 + n].rearrange("p k c -> p c k")
                    qi = q[:, :, 1, c0:c0 + n].rearrange("p k c -> p c k")
                    psP = ps.tile([H, NM], F32, tag="ps", name="psP")
                    nc.tensor.matmul(psP[:, 0:nf], lhsT=ghc, rhs=qr,
                                     start=True, stop=False)
                    nc.tensor.matmul(psP[:, 0:nf], lhsT=ghsn, rhs=qi,
                                     start=False, stop=True)
                    if t % 2 == 0:
                        nc.scalar.copy(
                            out=r4[:, c0:c0 + n, 0:KEEP],
                            in_=psP[:, 0:nf].rearrange("p (c k) -> p c k",
                                                       k=KEEP))
                    else:
                        nc.vector.tensor_copy(
                            out=r4[:, c0:c0 + n, 0:KEEP],
                            in_=psP[:, 0:nf].rearrange("p (c k) -> p c k",
                                                       k=KEEP))
                    psQ = ps.tile([H, NM], F32, tag="ps", name="psQ")
                    nc.tensor.matmul(psQ[:, 0:nf], lhsT=ghs, rhs=qr,
                                     start=True, stop=False)
                    nc.tensor.matmul(psQ[:, 0:nf], lhsT=ghc, rhs=qi,
                                     start=False, stop=True)
                    if t % 2 == 0:
                        nc.vector.tensor_copy(
                            out=r4[:, c0:c0 + n, KEEP:K2],
                            in_=psQ[:, 0:nf].rearrange("p (c k) -> p c k",
                                                       k=KEEP))
                    else:
                        nc.scalar.copy(
                            out=r4[:, c0:c0 + n, KEEP:K2],
                            in_=psQ[:, 0:nf].rearrange("p (c k) -> p c k",
                                                       k=KEEP))

            def s6():  # T4 bounce
                t4 = dr.tile([H, BS * 128], BF16, tag="t4", name="t4")
                nc.sync.dma_start(out=t4,
                                  in_=r4.rearrange("p c k -> p (c k)"))
                nc.sync.dma_start(out=xt4, in_=t4.rearrange("p f -> (p f)")
                                  .rearrange("(r c) -> r c", c=128),
                                  transpose=True)

            def s7():  # invW + residual + store (one DMA per (b, j))
                x1 = st["x1"]
                for j in range(2):
                    xres = x1[:, j].rearrange("p h c -> p (h c)")
                    yo = sb.tile([90, HC], BF16, tag="yo", name="yo",
                                 bufs=1)
                    for q3 in range(HC // 480):  # 18 chunks of 480
                        sl = slice(q3 * 480, (q3 + 1) * 480)
                        psW = ps.tile([90, 480], F32, tag="ps",
                                      name="psW")
                        nc.tensor.matmul(psW,
                                         lhsT=gw_t[:, j * 90:(j + 1) * 90],
                                         rhs=xt4[0:K2, sl],
                                         start=True, stop=True)
                        nc.vector.tensor_tensor(
                            out=yo[:, sl], in0=psW, in1=xres[:, sl],
                            op=ALU.add)
                    nc.sync.dma_start(out=yt[b, j], in_=yo)

            return [s0, s1, s2, s3, s4, s5, s6, s7]

        allst = [make_stages(b) for b in range(B)]
        SKEW = 3
        NSTAGE = 8
        for step in range(NSTAGE + SKEW * (B - 1)):
            for b in range(B):
                s = step - SKEW * b
                if 0 <= s < NSTAGE and (STAGE_MASK is None
                                        or s in STAGE_MASK):
                    allst[b][s]()

        for p in (dr, ps, sb, wpool):
            p.release()

    nc.compile()
    return nc


_NC = None


def _get_nc():
    global _NC
    if _NC is None:
        _NC = _build()
    return _NC


def _in_maps(x, w1, b1, w2, b2):
    fw, fhc, fhs, gw = _dft_mats()
    bf = ml_dtypes.bfloat16
    f8 = ml_dtypes.float8_e4m3
    fhb = np.stack([fhc, -fhs, fhs, fhc]).astype(f8)  # [C,-S,S,C] fused rhs
    fhg = np.stack([fhc, fhs, -fhs]).astype(bf)    # same matrices for inverse
    # x -> [B, w(90), j(2), h(90), C]
    xr = np.ascontiguousarray(
        x.reshape(B, H, 2, 90, C).transpose(0, 3, 2, 1, 4)).astype(bf)
    in_maps = []
    for i in range(NCORES):
        cs = slice(i * BS, (i + 1) * BS)
        mw1 = np.stack([w1[0, i], w1[1, i], -w1[1, i]]).transpose(1, 0, 2)
        w2a = np.zeros((BS + 1, 4, BS), np.float32)
        w2a[0:BS, 0] = w2[0, i]
        w2a[BS, 0] = b2[0, i]
        w2a[0:BS, 1] = w2[1, i]
        w2a[BS, 1] = b2[1, i]
        w2a[0:BS, 2] = -w2[1, i]
        w2a[0:BS, 3] = w2[0, i]
        bias1 = np.stack([b1[0, i], b1[1, i]], axis=1).astype(np.float32)
        in_maps.append({
            "xt": np.ascontiguousarray(xr[:, :, :, :, cs]),
            "fw": fw, "fhb": fhb, "fhg": fhg, "gw": gw,
            "mw1": mw1.astype(bf), "w2a": w2a.astype(f8), "bias1": bias1,
        })
    return in_maps


def _run(x, w1, b1, w2, b2, trace=False, tmpdir=None):
    nc = _get_nc()
    try:
        res = run_bass_kernel_spmd(nc, _in_maps(x, w1, b1, w2, b2),
                                   core_ids=list(range(NCORES)), trace=trace,
                                   tmpdir=tmpdir)
    except ModuleNotFoundError:
        res = run_bass_kernel_spmd(nc, _in_maps(x, w1, b1, w2, b2),
                                   core_ids=list(range(NCORES)), trace=False)
    outs = [r["yt"] for r in res.results]
    y = np.concatenate(outs, axis=-1)           # [B, 2, 90, 8640*ncores]
    y = y.reshape(B, 2, 90, NCORES, H, BS)      # [b, j, w, core, h, c]
    y = y.transpose(0, 4, 1, 2, 3, 5).reshape(B, H, W, C).astype(np.float32)
    return y, res


def kernel(x, w1, b1, w2, b2):
    y, _ = _run(np.asarray(x), np.asarray(w1), np.asarray(b1),
                np.asarray(w2), np.asarray(b2))
    return y


def _bench(x, w1, b1, w2, b2, iters=20, profile_dir=None):
    """Persistent-jit timing: returns (best_ns, avg_ns) per whole-NEFF run."""
    import time
    import jax
    from jax.sharding import Mesh, PartitionSpec, NamedSharding
    from jax.experimental.shard_map import shard_map
    from concourse.bass2jax import (_bass_exec_p, install_neuronx_cc_hook,
                                    partition_id_tensor)

    install_neuronx_cc_hook()
    nc = _get_nc()
    in_maps = _in_maps(x, w1, b1, w2, b2)

    in_names, out_names, out_avals, zero_outs = [], [], [], []
    for alloc in nc.m.functions[0].allocations:
        if not isinstance(alloc, mybir.MemoryLocationSet):
            continue
        name = alloc.memorylocations[0].name
        pname = nc.partition_id_tensor.name if nc.partition_id_tensor else None
        if alloc.kind == "ExternalInput":
            if name != pname:
                in_names.append(name)
        elif alloc.kind == "ExternalOutput":
            out_names.append(name)
            shape = tuple(alloc.tensor_shape)
            dtype = mybir.dt.np(alloc.dtype)
            out_avals.append(jax.core.ShapedArray(shape, dtype))
            zero_outs.append(np.zeros(shape, dtype))
    n_params = len(in_names)
    in_names_all = in_names + out_names
    if nc.partition_id_tensor is not None:
        in_names_all = in_names_all + [nc.partition_id_tensor.name]

    def _body(*args):
        operands = list(args)
        if nc.partition_id_tensor is not None:
            operands.append(partition_id_tensor())
        outs = _bass_exec_p.bind(
            *operands, out_avals=tuple(out_avals), in_names=tuple(in_names_all),
            out_names=tuple(out_names), lowering_input_output_aliases=(),
            sim_require_finite=True, sim_require_nnan=True, nc=nc)
        return tuple(outs)

    devices = jax.devices()[:NCORES]
    mesh = Mesh(np.asarray(devices), ("core",))
    in_specs = (PartitionSpec("core"),) * (n_params + len(out_names))
    out_specs = (PartitionSpec("core"),) * len(out_names)
    fn = jax.jit(shard_map(_body, mesh=mesh, in_specs=in_specs,
                           out_specs=out_specs, check_rep=False),
                 keep_unused=True)
    per_core = [[np.asarray(m[n]) for n in in_names] for m in in_maps]
    concat_in = [np.concatenate([per_core[c][i] for c in range(NCORES)], axis=0)
                 for i in range(n_params)]
    concat_zeros = [np.zeros((NCORES * z.shape[0], *z.shape[1:]), z.dtype)
                    for z in zero_outs]
    sh = NamedSharding(mesh, PartitionSpec("core"))
    dev_in = [jax.device_put(a, sh) for a in concat_in + concat_zeros]

    r = fn(*dev_in)
    jax.block_until_ready(r)
    r = fn(*dev_in)
    jax.block_until_ready(r)

    def chain_time(n):
        t0 = time.perf_counter()
        outs = None
        for _ in range(n):
            outs = fn(*dev_in)
        jax.block_until_ready(outs)
        return time.perf_counter() - t0
    chain_time(2)
    t_small = min(chain_time(2) for _ in range(8))
    t_big = min(chain_time(iters + 2) for _ in range(8))
    per = (t_big - t_small) / iters
    return int(per * 1e9), int(t_big / (iters + 2) * 1e9)


# revision 6
# speedup vs baseline: 1.2055x; 1.2055x over previous
"""AFNO2D Trainium kernel (v3 — contiguous layouts, batched DMA).

Same stage chain as v2 (ds-A, ds-B, ws-MLP1, ds-MLP2, ws-invH, XBAR-T4,
ws-invW) but every PSUM->SBUF evacuation is a contiguous copy (no
rearranged/transposed access patterns), DMAs are batched (1 load per
batch, 1 store per (batch, j)), and the softshrink is split
ACT-copy + DVE-clamp/sub to balance engines.

Layouts (per core, c=96 channels, b=4 batches):
  x1 [90w, 2j, 90h, 96c]                  input, resident for residual
  xB [90h, 96c, 92k2]                     after W-rfft (stage A)
  zM [96c, 2ri, 46k, 90u]                 after H-fft  (stage B)
  h1 [97c', 2ri, 46k, 90u]                after MLP1 (row 96 = ones)
  q  [90u, 92k2, 96c]                     after MLP2+softshrink
  r4 [90h', 96c, 128k2pad]                after invH
  xt4 [128k2, 8640(h'c)]                  after XBAR transpose
  yt [b, 2j, 90w, 8640(hc)]               output
"""

import sys

sys.path.insert(0, "/opt/trn_rl_repo")

import numpy as np
import ml_dtypes

import concourse.bass as bass
import concourse.mybir as mybir
import concourse.tile as tile
from concourse import bacc
from concourse.bass_utils import run_bass_kernel_spmd

BF16 = mybir.dt.bfloat16
F32 = mybir.dt.float32
FP8 = mybir.dt.float8e4
AF = mybir.ActivationFunctionType
ALU = mybir.AluOpType

# problem constants
B, H, W, C = 4, 90, 180, 768
NB, BS = 8, 96  # blocks, block size (= per-core channels)
KEEP = 46       # kept W-modes
K2 = 92         # 2*KEEP (re,im stacked)
LAM = 0.01      # softshrink lambda
UK = H * KEEP   # 4140 flat (k,u) free size
NM = 460        # MLP1 chunk
HC = H * BS     # 8640

NCORES = 8

# stage-isolation mask for benchmarking (set via kernel2.STAGE_MASK before
# first _get_nc(); None = all stages)
STAGE_MASK = None


def _dft_mats():
    """Host-side DFT matrices (float32, cast to bf16 at upload)."""
    w = np.arange(W)[:, None]
    k = np.arange(KEEP)[None, :]
    ang = 2.0 * np.pi * w * k / W
    fw = np.concatenate([np.cos(ang), -np.sin(ang)], axis=1) / np.sqrt(W)  # [180, 92]
    fw = fw.reshape(2, 90, K2).transpose(1, 0, 2)  # [90w, 2j, 92]

    h = np.arange(H)[:, None]
    u = np.arange(H)[None, :]
    angh = 2.0 * np.pi * h * u / H
    fhc = np.cos(angh) / np.sqrt(H)
    fhs = np.sin(angh) / np.sqrt(H)

    kk = np.arange(KEEP)[:, None]
    ww = np.arange(W)[None, :]
    angw = 2.0 * np.pi * kk * ww / W
    m = np.full((KEEP, 1), 2.0)
    m[0, 0] = 1.0
    gw = np.concatenate(
        [m * np.cos(angw), -m * np.sin(angw)], axis=0
    ) / np.sqrt(W)  # [92, 180]

    bf = ml_dtypes.bfloat16
    return (fw.astype(bf), fhc.astype(np.float32), fhs.astype(np.float32),
            gw.astype(bf))


def _build():
    nc = bacc.Bacc("TRN2", target_bir_lowering=False, debug=False,
                   num_devices=NCORES)

    # DRAM I/O (per core)
    xt = nc.dram_tensor("xt", [B, 90, 2, 90, BS], BF16,
                        kind="ExternalInput").ap()      # [b, w, j, h, c]
    fw_d = nc.dram_tensor("fw", [90, 2, K2], BF16, kind="ExternalInput").ap()
    fhb_d = nc.dram_tensor("fhb", [4, H, H], FP8, kind="ExternalInput").ap()
    fhg_d = nc.dram_tensor("fhg", [3, H, H], BF16, kind="ExternalInput").ap()
    gw_d = nc.dram_tensor("gw", [K2, W], BF16, kind="ExternalInput").ap()
    mw1_d = nc.dram_tensor("mw1", [BS, 3, BS], BF16, kind="ExternalInput").ap()
    w2a_d = nc.dram_tensor("w2a", [BS + 1, 4, BS], FP8,
                           kind="ExternalInput").ap()
    bias1_d = nc.dram_tensor("bias1", [BS, 2], F32, kind="ExternalInput").ap()
    yt = nc.dram_tensor("yt", [B, 2, 90, HC], BF16,
                        kind="ExternalOutput").ap()     # [b, j, w, (h c)]

    with tile.TileContext(nc) as tc:
        wpool = tc.alloc_tile_pool(name="w", bufs=1)
        sb = tc.alloc_tile_pool(name="sb", bufs=1)
        ps = tc.alloc_tile_pool(name="ps", bufs=8, space="PSUM")
        dr = tc.alloc_tile_pool(name="dr", bufs=2, space="DRAM")

        # ---- weights to SBUF (once) ----
        fw_t = wpool.tile([90, 2, K2], BF16, tag="fw")
        nc.gpsimd.dma_start(out=fw_t, in_=fw_d)
        fhb_t = wpool.tile([H, 4, H], FP8, tag="fhb")
        nc.gpsimd.dma_start(out=fhb_t, in_=fhb_d.rearrange("j p m -> p j m"))
        fhg_t = wpool.tile([H, 3, H], BF16, tag="fhg")
        nc.gpsimd.dma_start(out=fhg_t, in_=fhg_d.rearrange("j p m -> p j m"))
        gw_t = wpool.tile([K2, W], BF16, tag="gw")
        nc.gpsimd.dma_start(out=gw_t, in_=gw_d)
        mw1_t = wpool.tile([BS, 3, BS], BF16, tag="mw1")
        nc.gpsimd.dma_start(out=mw1_t, in_=mw1_d)
        w2a_t = wpool.tile([BS + 1, 4, BS], FP8, tag="w2a")
        nc.gpsimd.dma_start(out=w2a_t, in_=w2a_d)
        bias1_t = wpool.tile([BS, 2], F32, tag="bias1")
        nc.gpsimd.dma_start(out=bias1_t, in_=bias1_d)

        ghc, ghs, ghsn = fhg_t[:, 0], fhg_t[:, 1], fhg_t[:, 2]
        w1r, w1i, w1in = mw1_t[:, 0], mw1_t[:, 1], mw1_t[:, 2]

        # ---- persistent activation tiles ----
        h1 = wpool.tile([BS + 1, 2, KEEP, 128], FP8, tag="h1", name="h1")
        nc.gpsimd.memset(h1[:, :, :, H:128], 0.0)
        nc.gpsimd.memset(h1[BS:BS + 1, :, :, 0:H], 1.0)
        xBp = [wpool.tile([H, K2, 128], FP8, tag=f"xB{i}", name=f"xB{i}")
               for i in range(2)]
        for i in range(2):
            nc.gpsimd.memset(xBp[i][:, :, BS:128], 0.0)
        # r4: invH output, padded to 128 k2-cols for the XBAR transpose
        r4 = wpool.tile([H, BS, 128], BF16, tag="r4", name="r4")
        nc.gpsimd.memset(r4[:, :, K2:128], 0.0)
        xt4 = wpool.tile([128, HC], BF16, tag="xt4", name="xt4")

        def make_stages(b):
            st = {}

            def s0():  # load x (one DMA; resident for residual)
                x1 = st["x1"] = sb.tile([90, 2, 90, BS], BF16, tag="x1",
                                        name="x1", bufs=2)
                nc.sync.dma_start(out=x1, in_=xt[b])

            def s1():  # stage A: W-rfft (data-stationary) -> xB [h, k2, c]
                x1 = st["x1"]
                xB = st["xB"] = xBp[b % 2]
                for g in range(20):  # 20 groups of <=5 c
                    c0 = g * 5
                    n = min(5, BS - c0)
                    psA = ps.tile([H, 5, K2], F32, tag="ps", name="psA")
                    for i in range(n):
                        nc.tensor.matmul(psA[:, i, :],
                                         lhsT=x1[:, 0, :, c0 + i],
                                         rhs=fw_t[:, 0, :],
                                         start=True, stop=False)
                        nc.tensor.matmul(psA[:, i, :],
                                         lhsT=x1[:, 1, :, c0 + i],
                                         rhs=fw_t[:, 1, :],
                                         start=False, stop=True)
                    eng = nc.scalar if g % 2 == 0 else nc.vector
                    (eng.copy if g % 2 == 0 else eng.tensor_copy)(
                        out=xB[:, :, c0:c0 + n],
                        in_=psA[:, 0:n, :].rearrange("p c k -> p k c"))

            def s2():  # stage B: H-fft (data-stationary) -> zM [c, ri, k, u]
                xB = st["xB"]
                zM = st["zM"] = sb.tile([BS, KEEP, 2, H], BF16, tag="zM",
                                        name="zM")
                for g in range(KEEP // 2):  # 23 groups of 2 k
                    k0 = g * 2
                    psR = ps.tile([128, 2, 2 * H], F32, tag="ps",
                                  name="psR")
                    for i in range(2):
                        k = k0 + i
                        nc.tensor.matmul(psR[:, i, :], lhsT=xB[:, k, :],
                                         rhs=fhb_t[:, 0:2, :],
                                         start=True, stop=False)
                        nc.tensor.matmul(psR[:, i, :],
                                         lhsT=xB[:, KEEP + k, :],
                                         rhs=fhb_t[:, 2:4, :],
                                         start=False, stop=True)
                    if g % 2 == 0:
                        nc.scalar.copy(out=zM[:, k0:k0 + 2, :, :],
                                       in_=psR[0:BS, :, :])
                    else:
                        nc.vector.tensor_copy(out=zM[:, k0:k0 + 2, :, :],
                                              in_=psR[0:BS, :, :])

            def s3():  # MLP layer 1 (weight-stationary) -> h1 [97, ri, k, u]
                zM = st["zM"]
                for t in range(10):  # k-chunks of <=5 (46 = 9*5 + 1)
                    k0 = t * 5
                    nk = min(5, KEEP - k0)
                    zr = zM[:, k0:k0 + nk, 0, :]
                    zi = zM[:, k0:k0 + nk, 1, :]
                    p1r = ps.tile([BS, 5, H], F32, tag="ps", name="p1r")
                    p1i = ps.tile([BS, 5, H], F32, tag="ps", name="p1i")
                    nc.tensor.matmul(p1r[:, 0:nk, :], lhsT=w1r, rhs=zr,
                                     start=True, stop=False)
                    nc.tensor.matmul(p1i[:, 0:nk, :], lhsT=w1r, rhs=zi,
                                     start=True, stop=False)
                    nc.tensor.matmul(p1r[:, 0:nk, :], lhsT=w1in, rhs=zi,
                                     start=False, stop=True)
                    nc.tensor.matmul(p1i[:, 0:nk, :], lhsT=w1i, rhs=zr,
                                     start=False, stop=True)
                    nc.scalar.activation(h1[0:BS, 0, k0:k0 + nk, 0:H],
                                         p1r[:, 0:nk, :],
                                         AF.Relu, bias=bias1_t[:, 0:1])
                    nc.scalar.activation(h1[0:BS, 1, k0:k0 + nk, 0:H],
                                         p1i[:, 0:nk, :],
                                         AF.Relu, bias=bias1_t[:, 1:2])

            def s4():  # MLP layer 2 (data-stationary) + softshrink -> q
                q = st["q"] = sb.tile([H, KEEP, 2, BS], BF16, tag="q",
                                      name="q")
                for g in range(KEEP // 2):  # 23 groups of 2 k
                    k0 = g * 2
                    qp = ps.tile([128, 2, 2 * BS], F32, tag="ps",
                                 name="qp")
                    for i in range(2):
                        k = k0 + i
                        nc.tensor.matmul(qp[:, i, :], lhsT=h1[:, 0, k, :],
                                         rhs=w2a_t[:, 0:2, :],
                                         start=True, stop=False)
                        nc.tensor.matmul(qp[:, i, :], lhsT=h1[:, 1, k, :],
                                         rhs=w2a_t[:, 2:4, :],
                                         start=False, stop=True)
                    sq = sb.tile([H, 2, 2 * BS], BF16, tag="sq",
                                 name="sq", bufs=6)
                    nc.scalar.copy(out=sq, in_=qp[0:H, :, :])
                    tcl = sb.tile([H, 2, 2 * BS], BF16, tag="tcl",
                                  name="tcl", bufs=6)
                    nc.vector.tensor_scalar(
                        out=tcl, in0=sq,
                        scalar1=-LAM, scalar2=LAM,
                        op0=ALU.max, op1=ALU.min)
                    nc.vector.tensor_tensor(
                        out=q[:, k0:k0 + 2, :, :],
                        in0=sq, in1=tcl,
                        op=ALU.subtract)

            def s5():  # invH (weight-stationary) -> r4 [h', c, k2pad]
                q = st["q"]
                for t in range(10):  # c-groups of <=10
                    c0 = t * 10
                    n = min(10, BS - c0)
                    nf = n * KEEP
                    qr = q[:, :, 0, c0:c0 + n].rearrange("p k c -> p c k")
                    qi = q[:, :, 1, c0:c0 + n].rearrange("p k c -> p c k")
                    psP = ps.tile([H, NM], F32, tag="ps", name="psP")
                    nc.tensor.matmul(psP[:, 0:nf], lhsT=ghc, rhs=qr,
                                     start=True, stop=False)
                    nc.tensor.matmul(psP[:, 0:nf], lhsT=ghsn, rhs=qi,
                                     start=False, stop=True)
                    if t % 2 == 0:
                        nc.scalar.copy(
                            out=r4[:, c0:c0 + n, 0:KEEP],
                            in_=psP[:, 0:nf].rearrange("p (c k) -> p c k",
                                                       k=KEEP))
                    else:
                        nc.vector.tensor_copy(
                            out=r4[:, c0:c0 + n, 0:KEEP],
                            in_=psP[:, 0:nf].rearrange("p (c k) -> p c k",
                                                       k=KEEP))
                    psQ = ps.tile([H, NM], F32, tag="ps", name="psQ")
                    nc.tensor.matmul(psQ[:, 0:nf], lhsT=ghs, rhs=qr,
                                     start=True, stop=False)
                    nc.tensor.matmul(psQ[:, 0:nf], lhsT=ghc, rhs=qi,
                                     start=False, stop=True)
                    if t % 2 == 0:
                        nc.vector.tensor_copy(
                            out=r4[:, c0:c0 + n, KEEP:K2],
                            in_=psQ[:, 0:nf].rearrange("p (c k) -> p c k",
                                                       k=KEEP))
                    else:
                        nc.scalar.copy(
                            out=r4[:, c0:c0 + n, KEEP:K2],
                            in_=psQ[:, 0:nf].rearrange("p (c k) -> p c k",
                                                       k=KEEP))

            def s6():  # T4 bounce
                t4 = dr.tile([H, BS * 128], BF16, tag="t4", name="t4")
                nc.sync.dma_start(out=t4,
                                  in_=r4.rearrange("p c k -> p (c k)"))
                nc.sync.dma_start(out=xt4, in_=t4.rearrange("p f -> (p f)")
                                  .rearrange("(r c) -> r c", c=128),
                                  transpose=True)

            def s7():  # invW + residual + store (one DMA per (b, j))
                x1 = st["x1"]
                for j in range(2):
                    xres = x1[:, j].rearrange("p h c -> p (h c)")
                    yo = sb.tile([90, HC], BF16, tag="yo", name="yo",
                                 bufs=1)
                    for q3 in range(HC // 480):  # 18 chunks of 480
                        sl = slice(q3 * 480, (q3 + 1) * 480)
                        psW = ps.tile([90, 480], F32, tag="ps",
                                      name="psW")
                        nc.tensor.matmul(psW,
                                         lhsT=gw_t[:, j * 90:(j + 1) * 90],
                                         rhs=xt4[0:K2, sl],
                                         start=True, stop=True)
                        nc.vector.tensor_tensor(
                            out=yo[:, sl], in0=psW, in1=xres[:, sl],
                            op=ALU.add)
                    nc.sync.dma_start(out=yt[b, j], in_=yo)

            return [s0, s1, s2, s3, s4, s5, s6, s7]

        allst = [make_stages(b) for b in range(B)]
        SKEW = 3
        NSTAGE = 8
        for step in range(NSTAGE + SKEW * (B - 1)):
            for b in range(B):
                s = step - SKEW * b
                if 0 <= s < NSTAGE and (STAGE_MASK is None
                                        or s in STAGE_MASK):
                    allst[b][s]()

        for p in (dr, ps, sb, wpool):
            p.release()

    nc.compile()
    return nc


_NC = None


def _get_nc():
    global _NC
    if _NC is None:
        _NC = _build()
    return _NC


def _in_maps(x, w1, b1, w2, b2):
    fw, fhc, fhs, gw = _dft_mats()
    bf = ml_dtypes.bfloat16
    f8 = ml_dtypes.float8_e4m3
    fhb = np.stack([fhc, -fhs, fhs, fhc]).astype(f8)  # [C,-S,S,C] fused rhs
    fhg = np.stack([fhc, fhs, -fhs]).astype(bf)    # same matrices for inverse
    # x -> [B, w(90), j(2), h(90), C]
    xr = np.ascontiguousarray(
        x.reshape(B, H, 2, 90, C).transpose(0, 3, 2, 1, 4)).astype(bf)
    in_maps = []
    for i in range(NCORES):
        cs = slice(i * BS, (i + 1) * BS)
        mw1 = np.stack([w1[0, i], w1[1, i], -w1[1, i]]).transpose(1, 0, 2)
        w2a = np.zeros((BS + 1, 4, BS), np.float32)
        w2a[0:BS, 0] = w2[0, i]
        w2a[BS, 0] = b2[0, i]
        w2a[0:BS, 1] = w2[1, i]
        w2a[BS, 1] = b2[1, i]
        w2a[0:BS, 2] = -w2[1, i]
        w2a[0:BS, 3] = w2[0, i]
        bias1 = np.stack([b1[0, i], b1[1, i]], axis=1).astype(np.float32)
        in_maps.append({
            "xt": np.ascontiguousarray(xr[:, :, :, :, cs]),
            "fw": fw, "fhb": fhb, "fhg": fhg, "gw": gw,
            "mw1": mw1.astype(bf), "w2a": w2a.astype(f8), "bias1": bias1,
        })
    return in_maps


def _run(x, w1, b1, w2, b2, trace=False, tmpdir=None):
    nc = _get_nc()
    try:
        res = run_bass_kernel_spmd(nc, _in_maps(x, w1, b1, w2, b2),
                                   core_ids=list(range(NCORES)), trace=trace,
                                   tmpdir=tmpdir)
    except ModuleNotFoundError:
        res = run_bass_kernel_spmd(nc, _in_maps(x, w1, b1, w2, b2),
                                   core_ids=list(range(NCORES)), trace=False)
    outs = [r["yt"] for r in res.results]
    y = np.concatenate(outs, axis=-1)           # [B, 2, 90, 8640*ncores]
    y = y.reshape(B, 2, 90, NCORES, H, BS)      # [b, j, w, core, h, c]
    y = y.transpose(0, 4, 1, 2, 3, 5).reshape(B, H, W, C).astype(np.float32)
    return y, res


def kernel(x, w1, b1, w2, b2):
    y, _ = _run(np.asarray(x), np.asarray(w1), np.asarray(b1),
                np.asarray(w2), np.asarray(b2))
    return y


def _bench(x, w1, b1, w2, b2, iters=20, profile_dir=None):
    """Persistent-jit timing: returns (best_ns, avg_ns) per whole-NEFF run."""
    import time
    import jax
    from jax.sharding import Mesh, PartitionSpec, NamedSharding
    from jax.experimental.shard_map import shard_map
    from concourse.bass2jax import (_bass_exec_p, install_neuronx_cc_hook,
                                    partition_id_tensor)

    install_neuronx_cc_hook()
    nc = _get_nc()
    in_maps = _in_maps(x, w1, b1, w2, b2)

    in_names, out_names, out_avals, zero_outs = [], [], [], []
    for alloc in nc.m.functions[0].allocations:
        if not isinstance(alloc, mybir.MemoryLocationSet):
            continue
        name = alloc.memorylocations[0].name
        pname = nc.partition_id_tensor.name if nc.partition_id_tensor else None
        if alloc.kind == "ExternalInput":
            if name != pname:
                in_names.append(name)
        elif alloc.kind == "ExternalOutput":
            out_names.append(name)
            shape = tuple(alloc.tensor_shape)
            dtype = mybir.dt.np(alloc.dtype)
            out_avals.append(jax.core.ShapedArray(shape, dtype))
            zero_outs.append(np.zeros(shape, dtype))
    n_params = len(in_names)
    in_names_all = in_names + out_names
    if nc.partition_id_tensor is not None:
        in_names_all = in_names_all + [nc.partition_id_tensor.name]

    def _body(*args):
        operands = list(args)
        if nc.partition_id_tensor is not None:
            operands.append(partition_id_tensor())
        outs = _bass_exec_p.bind(
            *operands, out_avals=tuple(out_avals), in_names=tuple(in_names_all),
            out_names=tuple(out_names), lowering_input_output_aliases=(),
            sim_require_finite=True, sim_require_nnan=True, nc=nc)
        return tuple(outs)

    devices = jax.devices()[:NCORES]
    mesh = Mesh(np.asarray(devices), ("core",))
    in_specs = (PartitionSpec("core"),) * (n_params + len(out_names))
    out_specs = (PartitionSpec("core"),) * len(out_names)
    fn = jax.jit(shard_map(_body, mesh=mesh, in_specs=in_specs,
                           out_specs=out_specs, check_rep=False),
                 keep_unused=True)
    per_core = [[np.asarray(m[n]) for n in in_names] for m in in_maps]
    concat_in = [np.concatenate([per_core[c][i] for c in range(NCORES)], axis=0)
                 for i in range(n_params)]
    concat_zeros = [np.zeros((NCORES * z.shape[0], *z.shape[1:]), z.dtype)
                    for z in zero_outs]
    sh = NamedSharding(mesh, PartitionSpec("core"))
    dev_in = [jax.device_put(a, sh) for a in concat_in + concat_zeros]

    r = fn(*dev_in)
    jax.block_until_ready(r)
    r = fn(*dev_in)
    jax.block_until_ready(r)

    def chain_time(n):
        t0 = time.perf_counter()
        outs = None
        for _ in range(n):
            outs = fn(*dev_in)
        jax.block_until_ready(outs)
        return time.perf_counter() - t0
    chain_time(2)
    t_small = min(chain_time(2) for _ in range(8))
    t_big = min(chain_time(iters + 2) for _ in range(8))
    per = (t_big - t_small) / iters
    return int(per * 1e9), int(t_big / (iters + 2) * 1e9)


# revision 7
# speedup vs baseline: 1.4461x; 1.1996x over previous
"""AFNO2D Trainium kernel (v3 — contiguous layouts, batched DMA).

Same stage chain as v2 (ds-A, ds-B, ws-MLP1, ds-MLP2, ws-invH, XBAR-T4,
ws-invW) but every PSUM->SBUF evacuation is a contiguous copy (no
rearranged/transposed access patterns), DMAs are batched (1 load per
batch, 1 store per (batch, j)), and the softshrink is split
ACT-copy + DVE-clamp/sub to balance engines.

Layouts (per core, c=96 channels, b=4 batches):
  x1 [90w, 2j, 90h, 96c]                  input, resident for residual
  xB [90h, 96c, 92k2]                     after W-rfft (stage A)
  zM [96c, 2ri, 46k, 90u]                 after H-fft  (stage B)
  h1 [97c', 2ri, 46k, 90u]                after MLP1 (row 96 = ones)
  q  [90u, 92k2, 96c]                     after MLP2+softshrink
  r4 [90h', 96c, 128k2pad]                after invH
  xt4 [128k2, 8640(h'c)]                  after XBAR transpose
  yt [b, 2j, 90w, 8640(hc)]               output
"""

import sys

sys.path.insert(0, "/opt/trn_rl_repo")

import numpy as np
import ml_dtypes

import concourse.bass as bass
import concourse.mybir as mybir
import concourse.tile as tile
from concourse import bacc
from concourse.bass_utils import run_bass_kernel_spmd

BF16 = mybir.dt.bfloat16
F32 = mybir.dt.float32
FP8 = mybir.dt.float8e4
AF = mybir.ActivationFunctionType
ALU = mybir.AluOpType

# problem constants
B, H, W, C = 4, 90, 180, 768
NB, BS = 8, 96  # blocks, block size (= per-core channels)
KEEP = 46       # kept W-modes
K2 = 92         # 2*KEEP (re,im stacked)
LAM = 0.01      # softshrink lambda
UK = H * KEEP   # 4140 flat (k,u) free size
NM = 460        # MLP1 chunk
HC = H * BS     # 8640

NCORES = 8

# stage-isolation mask for benchmarking (set via kernel2.STAGE_MASK before
# first _get_nc(); None = all stages)
STAGE_MASK = None


def _dft_mats():
    """Host-side DFT matrices (float32, cast to bf16 at upload)."""
    w = np.arange(W)[:, None]
    k = np.arange(KEEP)[None, :]
    ang = 2.0 * np.pi * w * k / W
    fw = np.concatenate([np.cos(ang), -np.sin(ang)], axis=1) / np.sqrt(W)  # [180, 92]
    fw = fw.reshape(2, 90, K2).transpose(1, 0, 2)  # [90w, 2j, 92]

    h = np.arange(H)[:, None]
    u = np.arange(H)[None, :]
    angh = 2.0 * np.pi * h * u / H
    fhc = np.cos(angh) / np.sqrt(H)
    fhs = np.sin(angh) / np.sqrt(H)

    kk = np.arange(KEEP)[:, None]
    ww = np.arange(W)[None, :]
    angw = 2.0 * np.pi * kk * ww / W
    m = np.full((KEEP, 1), 2.0)
    m[0, 0] = 1.0
    gw = np.concatenate(
        [m * np.cos(angw), -m * np.sin(angw)], axis=0
    ) / np.sqrt(W)  # [92, 180]

    bf = ml_dtypes.bfloat16
    return (fw.astype(bf), fhc.astype(np.float32), fhs.astype(np.float32),
            gw.astype(bf))


def _build():
    nc = bacc.Bacc("TRN2", target_bir_lowering=False, debug=False,
                   num_devices=NCORES)

    # DRAM I/O (per core)
    xt = nc.dram_tensor("xt", [B, 90, 2, 90, BS], BF16,
                        kind="ExternalInput").ap()      # [b, w, j, h, c]
    fw_d = nc.dram_tensor("fw", [90, 2, K2], BF16, kind="ExternalInput").ap()
    fhb_d = nc.dram_tensor("fhb", [4, H, H], FP8, kind="ExternalInput").ap()
    fhg_d = nc.dram_tensor("fhg", [3, H, H], BF16, kind="ExternalInput").ap()
    gw_d = nc.dram_tensor("gw", [K2, W], BF16, kind="ExternalInput").ap()
    mw1_d = nc.dram_tensor("mw1", [BS, 3, BS], BF16, kind="ExternalInput").ap()
    w2a_d = nc.dram_tensor("w2a", [BS + 1, 4, BS], FP8,
                           kind="ExternalInput").ap()
    bias1_d = nc.dram_tensor("bias1", [BS, 2], F32, kind="ExternalInput").ap()
    yt = nc.dram_tensor("yt", [B, 2, 90, HC], BF16,
                        kind="ExternalOutput").ap()     # [b, j, w, (h c)]

    with tile.TileContext(nc) as tc:
        wpool = tc.alloc_tile_pool(name="w", bufs=1)
        sb = tc.alloc_tile_pool(name="sb", bufs=1)
        ps = tc.alloc_tile_pool(name="ps", bufs=8, space="PSUM")
        dr = tc.alloc_tile_pool(name="dr", bufs=2, space="DRAM")

        # ---- weights to SBUF (once) ----
        fw_t = wpool.tile([90, 2, K2], BF16, tag="fw")
        nc.gpsimd.dma_start(out=fw_t, in_=fw_d)
        fhb_t = wpool.tile([H, 4, H], FP8, tag="fhb")
        nc.gpsimd.dma_start(out=fhb_t, in_=fhb_d.rearrange("j p m -> p j m"))
        fhg_t = wpool.tile([H, 3, H], BF16, tag="fhg")
        nc.gpsimd.dma_start(out=fhg_t, in_=fhg_d.rearrange("j p m -> p j m"))
        gw_t = wpool.tile([K2, W], BF16, tag="gw")
        nc.gpsimd.dma_start(out=gw_t, in_=gw_d)
        mw1_t = wpool.tile([BS, 3, BS], BF16, tag="mw1")
        nc.gpsimd.dma_start(out=mw1_t, in_=mw1_d)
        w2a_t = wpool.tile([BS + 1, 4, BS], FP8, tag="w2a")
        nc.gpsimd.dma_start(out=w2a_t, in_=w2a_d)
        bias1_t = wpool.tile([BS, 2], F32, tag="bias1")
        nc.gpsimd.dma_start(out=bias1_t, in_=bias1_d)

        ghc, ghs, ghsn = fhg_t[:, 0], fhg_t[:, 1], fhg_t[:, 2]
        w1r, w1i, w1in = mw1_t[:, 0], mw1_t[:, 1], mw1_t[:, 2]

        # ---- persistent activation tiles ----
        h1 = wpool.tile([BS + 1, 2, KEEP, 128], FP8, tag="h1", name="h1")
        nc.gpsimd.memset(h1[:, :, :, H:128], 0.0)
        nc.gpsimd.memset(h1[BS:BS + 1, :, :, 0:H], 1.0)
        xBp = [wpool.tile([H, K2, 128], FP8, tag=f"xB{i}", name=f"xB{i}")
               for i in range(2)]
        for i in range(2):
            nc.gpsimd.memset(xBp[i][:, :, BS:128], 0.0)
        # r4: invH output, padded to 128 k2-cols for the XBAR transpose
        r4 = wpool.tile([H, BS, 128], BF16, tag="r4", name="r4")
        nc.gpsimd.memset(r4[:, :, K2:128], 0.0)
        xt4 = wpool.tile([128, HC], BF16, tag="xt4", name="xt4")

        def make_stages(b):
            st = {}

            def s0():  # load x (one DMA; resident for residual)
                x1 = st["x1"] = sb.tile([90, 2, 90, BS], BF16, tag="x1",
                                        name="x1", bufs=2)
                nc.sync.dma_start(out=x1, in_=xt[b])

            def s1():  # stage A: W-rfft (data-stationary) -> xB [h, k2, c]
                x1 = st["x1"]
                xB = st["xB"] = xBp[b % 2]
                for g in range(20):  # 20 groups of <=5 c
                    c0 = g * 5
                    n = min(5, BS - c0)
                    psA = ps.tile([H, 5, K2], F32, tag="ps", name="psA")
                    for i in range(n):
                        nc.tensor.matmul(psA[:, i, :],
                                         lhsT=x1[:, 0, :, c0 + i],
                                         rhs=fw_t[:, 0, :],
                                         start=True, stop=False)
                        nc.tensor.matmul(psA[:, i, :],
                                         lhsT=x1[:, 1, :, c0 + i],
                                         rhs=fw_t[:, 1, :],
                                         start=False, stop=True)
                    eng = nc.scalar if g % 2 == 0 else nc.vector
                    (eng.copy if g % 2 == 0 else eng.tensor_copy)(
                        out=xB[:, :, c0:c0 + n],
                        in_=psA[:, 0:n, :].rearrange("p c k -> p k c"))

            def s2():  # stage B: H-fft (data-stationary) -> zM [c, ri, k, u]
                xB = st["xB"]
                zM = st["zM"] = sb.tile([BS, KEEP, 2, H], BF16, tag="zM",
                                        name="zM")
                for g in range(KEEP // 2):  # 23 groups of 2 k
                    k0 = g * 2
                    psR = ps.tile([128, 2, 2 * H], F32, tag="ps",
                                  name="psR")
                    for i in range(2):
                        k = k0 + i
                        nc.tensor.matmul(psR[:, i, :], lhsT=xB[:, k, :],
                                         rhs=fhb_t[:, 0:2, :],
                                         start=True, stop=False)
                        nc.tensor.matmul(psR[:, i, :],
                                         lhsT=xB[:, KEEP + k, :],
                                         rhs=fhb_t[:, 2:4, :],
                                         start=False, stop=True)
                    if g % 2 == 0:
                        nc.scalar.copy(out=zM[:, k0:k0 + 2, :, :],
                                       in_=psR[0:BS, :, :])
                    else:
                        nc.vector.tensor_copy(out=zM[:, k0:k0 + 2, :, :],
                                              in_=psR[0:BS, :, :])

            def s3():  # MLP layer 1 (weight-stationary) -> h1 [97, ri, k, u]
                zM = st["zM"]
                for t in range(10):  # k-chunks of <=5 (46 = 9*5 + 1)
                    k0 = t * 5
                    nk = min(5, KEEP - k0)
                    zr = zM[:, k0:k0 + nk, 0, :]
                    zi = zM[:, k0:k0 + nk, 1, :]
                    p1r = ps.tile([BS, 5, H], F32, tag="ps", name="p1r")
                    p1i = ps.tile([BS, 5, H], F32, tag="ps", name="p1i")
                    nc.tensor.matmul(p1r[:, 0:nk, :], lhsT=w1r, rhs=zr,
                                     start=True, stop=False)
                    nc.tensor.matmul(p1i[:, 0:nk, :], lhsT=w1r, rhs=zi,
                                     start=True, stop=False)
                    nc.tensor.matmul(p1r[:, 0:nk, :], lhsT=w1in, rhs=zi,
                                     start=False, stop=True)
                    nc.tensor.matmul(p1i[:, 0:nk, :], lhsT=w1i, rhs=zr,
                                     start=False, stop=True)
                    nc.scalar.activation(h1[0:BS, 0, k0:k0 + nk, 0:H],
                                         p1r[:, 0:nk, :],
                                         AF.Relu, bias=bias1_t[:, 0:1])
                    nc.scalar.activation(h1[0:BS, 1, k0:k0 + nk, 0:H],
                                         p1i[:, 0:nk, :],
                                         AF.Relu, bias=bias1_t[:, 1:2])

            def s4():  # MLP layer 2 (data-stationary) + softshrink -> q
                q = st["q"] = sb.tile([H, KEEP, 2, BS], BF16, tag="q",
                                      name="q")
                for g in range(KEEP // 2):  # 23 groups of 2 k
                    k0 = g * 2
                    qp = ps.tile([128, 2, 2 * BS], F32, tag="ps",
                                 name="qp")
                    for i in range(2):
                        k = k0 + i
                        nc.tensor.matmul(qp[:, i, :], lhsT=h1[:, 0, k, :],
                                         rhs=w2a_t[:, 0:2, :],
                                         start=True, stop=False)
                        nc.tensor.matmul(qp[:, i, :], lhsT=h1[:, 1, k, :],
                                         rhs=w2a_t[:, 2:4, :],
                                         start=False, stop=True)
                    sq = sb.tile([H, 2, 2 * BS], BF16, tag="sq",
                                 name="sq", bufs=6)
                    nc.scalar.copy(out=sq, in_=qp[0:H, :, :])
                    tcl = sb.tile([H, 2, 2 * BS], BF16, tag="tcl",
                                  name="tcl", bufs=6)
                    nc.vector.tensor_scalar(
                        out=tcl, in0=sq,
                        scalar1=-LAM, scalar2=LAM,
                        op0=ALU.max, op1=ALU.min)
                    nc.vector.tensor_tensor(
                        out=q[:, k0:k0 + 2, :, :],
                        in0=sq, in1=tcl,
                        op=ALU.subtract)

            def s5():  # invH (weight-stationary) -> r4 [h', c, k2pad]
                q = st["q"]
                for t in range(10):  # c-groups of <=10
                    c0 = t * 10
                    n = min(10, BS - c0)
                    nf = n * KEEP
                    qr = q[:, :, 0, c0:c0 + n].rearrange("p k c -> p c k")
                    qi = q[:, :, 1, c0:c0 + n].rearrange("p k c -> p c k")
                    psP = ps.tile([H, NM], F32, tag="ps", name="psP")
                    nc.tensor.matmul(psP[:, 0:nf], lhsT=ghc, rhs=qr,
                                     start=True, stop=False)
                    nc.tensor.matmul(psP[:, 0:nf], lhsT=ghsn, rhs=qi,
                                     start=False, stop=True)
                    if t % 2 == 0:
                        nc.scalar.copy(
                            out=r4[:, c0:c0 + n, 0:KEEP],
                            in_=psP[:, 0:nf].rearrange("p (c k) -> p c k",
                                                       k=KEEP))
                    else:
                        nc.vector.tensor_copy(
                            out=r4[:, c0:c0 + n, 0:KEEP],
                            in_=psP[:, 0:nf].rearrange("p (c k) -> p c k",
                                                       k=KEEP))
                    psQ = ps.tile([H, NM], F32, tag="ps", name="psQ")
                    nc.tensor.matmul(psQ[:, 0:nf], lhsT=ghs, rhs=qr,
                                     start=True, stop=False)
                    nc.tensor.matmul(psQ[:, 0:nf], lhsT=ghc, rhs=qi,
                                     start=False, stop=True)
                    if t % 2 == 0:
                        nc.vector.tensor_copy(
                            out=r4[:, c0:c0 + n, KEEP:K2],
                            in_=psQ[:, 0:nf].rearrange("p (c k) -> p c k",
                                                       k=KEEP))
                    else:
                        nc.scalar.copy(
                            out=r4[:, c0:c0 + n, KEEP:K2],
                            in_=psQ[:, 0:nf].rearrange("p (c k) -> p c k",
                                                       k=KEEP))

            def s6():  # T4 bounce, transpose-load split in halves
                t4 = dr.tile([H, BS * 128], BF16, tag="t4", name="t4")
                nc.sync.dma_start(out=t4,
                                  in_=r4.rearrange("p c k -> p (c k)"))
                t4r = (t4.rearrange("p f -> (p f)")
                       .rearrange("(r c) -> r c", c=128))
                nc.sync.dma_start(out=xt4[:, 0:HC // 2],
                                  in_=t4r[0:HC // 2, :], transpose=True)
                nc.sync.dma_start(out=xt4[:, HC // 2:HC],
                                  in_=t4r[HC // 2:HC, :], transpose=True)

            def s7():  # invW + residual + store (one DMA per (b, j))
                x1 = st["x1"]
                for j in range(2):
                    xres = x1[:, j].rearrange("p h c -> p (h c)")
                    yo = sb.tile([90, HC], BF16, tag="yo", name="yo",
                                 bufs=1)
                    for q3 in range(HC // 480):  # 18 chunks of 480
                        sl = slice(q3 * 480, (q3 + 1) * 480)
                        psW = ps.tile([90, 480], F32, tag="ps",
                                      name="psW")
                        nc.tensor.matmul(psW,
                                         lhsT=gw_t[:, j * 90:(j + 1) * 90],
                                         rhs=xt4[0:K2, sl],
                                         start=True, stop=True)
                        nc.vector.tensor_tensor(
                            out=yo[:, sl], in0=psW, in1=xres[:, sl],
                            op=ALU.add)
                    nc.sync.dma_start(out=yt[b, j], in_=yo)

            return [s0, s1, s2, s3, s4, s5, s6, s7]

        allst = [make_stages(b) for b in range(B)]
        SKEW = 3
        NSTAGE = 8
        for step in range(NSTAGE + SKEW * (B - 1)):
            for b in range(B):
                s = step - SKEW * b
                if 0 <= s < NSTAGE and (STAGE_MASK is None
                                        or s in STAGE_MASK):
                    allst[b][s]()

        for p in (dr, ps, sb, wpool):
            p.release()

    nc.compile()
    return nc


_NC = None


def _get_nc():
    global _NC
    if _NC is None:
        _NC = _build()
    return _NC


def _in_maps(x, w1, b1, w2, b2):
    fw, fhc, fhs, gw = _dft_mats()
    bf = ml_dtypes.bfloat16
    f8 = ml_dtypes.float8_e4m3
    fhb = np.stack([fhc, -fhs, fhs, fhc]).astype(f8)  # [C,-S,S,C] fused rhs
    fhg = np.stack([fhc, fhs, -fhs]).astype(bf)    # same matrices for inverse
    # x -> [B, w(90), j(2), h(90), C]
    xr = np.ascontiguousarray(
        x.reshape(B, H, 2, 90, C).transpose(0, 3, 2, 1, 4)).astype(bf)
    in_maps = []
    for i in range(NCORES):
        cs = slice(i * BS, (i + 1) * BS)
        mw1 = np.stack([w1[0, i], w1[1, i], -w1[1, i]]).transpose(1, 0, 2)
        w2a = np.zeros((BS + 1, 4, BS), np.float32)
        w2a[0:BS, 0] = w2[0, i]
        w2a[BS, 0] = b2[0, i]
        w2a[0:BS, 1] = w2[1, i]
        w2a[BS, 1] = b2[1, i]
        w2a[0:BS, 2] = -w2[1, i]
        w2a[0:BS, 3] = w2[0, i]
        bias1 = np.stack([b1[0, i], b1[1, i]], axis=1).astype(np.float32)
        in_maps.append({
            "xt": np.ascontiguousarray(xr[:, :, :, :, cs]),
            "fw": fw, "fhb": fhb, "fhg": fhg, "gw": gw,
            "mw1": mw1.astype(bf), "w2a": w2a.astype(f8), "bias1": bias1,
        })
    return in_maps


def _run(x, w1, b1, w2, b2, trace=False, tmpdir=None):
    nc = _get_nc()
    try:
        res = run_bass_kernel_spmd(nc, _in_maps(x, w1, b1, w2, b2),
                                   core_ids=list(range(NCORES)), trace=trace,
                                   tmpdir=tmpdir)
    except ModuleNotFoundError:
        res = run_bass_kernel_spmd(nc, _in_maps(x, w1, b1, w2, b2),
                                   core_ids=list(range(NCORES)), trace=False)
    outs = [r["yt"] for r in res.results]
    y = np.concatenate(outs, axis=-1)           # [B, 2, 90, 8640*ncores]
    y = y.reshape(B, 2, 90, NCORES, H, BS)      # [b, j, w, core, h, c]
    y = y.transpose(0, 4, 1, 2, 3, 5).reshape(B, H, W, C).astype(np.float32)
    return y, res


def kernel(x, w1, b1, w2, b2):
    y, _ = _run(np.asarray(x), np.asarray(w1), np.asarray(b1),
                np.asarray(w2), np.asarray(b2))
    return y


def _bench(x, w1, b1, w2, b2, iters=20, profile_dir=None):
    """Persistent-jit timing: returns (best_ns, avg_ns) per whole-NEFF run."""
    import time
    import jax
    from jax.sharding import Mesh, PartitionSpec, NamedSharding
    from jax.experimental.shard_map import shard_map
    from concourse.bass2jax import (_bass_exec_p, install_neuronx_cc_hook,
                                    partition_id_tensor)

    install_neuronx_cc_hook()
    nc = _get_nc()
    in_maps = _in_maps(x, w1, b1, w2, b2)

    in_names, out_names, out_avals, zero_outs = [], [], [], []
    for alloc in nc.m.functions[0].allocations:
        if not isinstance(alloc, mybir.MemoryLocationSet):
            continue
        name = alloc.memorylocations[0].name
        pname = nc.partition_id_tensor.name if nc.partition_id_tensor else None
        if alloc.kind == "ExternalInput":
            if name != pname:
                in_names.append(name)
        elif alloc.kind == "ExternalOutput":
            out_names.append(name)
            shape = tuple(alloc.tensor_shape)
            dtype = mybir.dt.np(alloc.dtype)
            out_avals.append(jax.core.ShapedArray(shape, dtype))
            zero_outs.append(np.zeros(shape, dtype))
    n_params = len(in_names)
    in_names_all = in_names + out_names
    if nc.partition_id_tensor is not None:
        in_names_all = in_names_all + [nc.partition_id_tensor.name]

    def _body(*args):
        operands = list(args)
        if nc.partition_id_tensor is not None:
            operands.append(partition_id_tensor())
        outs = _bass_exec_p.bind(
            *operands, out_avals=tuple(out_avals), in_names=tuple(in_names_all),
            out_names=tuple(out_names), lowering_input_output_aliases=(),
            sim_require_finite=True, sim_require_nnan=True, nc=nc)
        return tuple(outs)

    devices = jax.devices()[:NCORES]
    mesh = Mesh(np.asarray(devices), ("core",))
    in_specs = (PartitionSpec("core"),) * (n_params + len(out_names))
    out_specs = (PartitionSpec("core"),) * len(out_names)
    fn = jax.jit(shard_map(_body, mesh=mesh, in_specs=in_specs,
                           out_specs=out_specs, check_rep=False),
                 keep_unused=True)
    per_core = [[np.asarray(m[n]) for n in in_names] for m in in_maps]
    concat_in = [np.concatenate([per_core[c][i] for c in range(NCORES)], axis=0)
                 for i in range(n_params)]
    concat_zeros = [np.zeros((NCORES * z.shape[0], *z.shape[1:]), z.dtype)
                    for z in zero_outs]
    sh = NamedSharding(mesh, PartitionSpec("core"))
    dev_in = [jax.device_put(a, sh) for a in concat_in + concat_zeros]

    r = fn(*dev_in)
    jax.block_until_ready(r)
    r = fn(*dev_in)
    jax.block_until_ready(r)

    def chain_time(n):
        t0 = time.perf_counter()
        outs = None
        for _ in range(n):
            outs = fn(*dev_in)
        jax.block_until_ready(outs)
        return time.perf_counter() - t0
    chain_time(2)
    t_small = min(chain_time(2) for _ in range(8))
    t_big = min(chain_time(iters + 2) for _ in range(8))
    per = (t_big - t_small) / iters
    return int(per * 1e9), int(t_big / (iters + 2) * 1e9)
